# revision 7
# baseline (speedup 1.0000x reference)
"""Trainium2 Bass kernel for nn_Attention_558345749040.

Reference computation (per batch b, H=8 heads of d=64, S=4096, E=512):
    Q = Q_seq @ WQ ; K = K_seq @ WK ; V = V_seq @ WV      (per-token matmuls)
    A = (Q * K) / 8                                        (elementwise)
    A += -1e12 at head positions j >= V_len[b]             (additive mask)
    softmax over each head's 64-wide feature group
    O = softmax * V, rows s >= Q_len[b] zeroed

Sharding: pure data parallel, batch b -> core b (B == 8 == n_cores).

Device algorithm (per core, per 128-token chunk, token-major tiles):
  Q/K projections in float32r (full-rate PE, ~2^-13 effective precision),
  V projection in fp16. Host pre-zeroes the masked columns of WK and WV, so
  masked positions have K=0 => logits A_j = 0 exactly. The group max over
  raw logits is then >= 0 and >= every unmasked logit, which makes the
  mask-free max safe: exp(A - M) <= 1, masked exp contributes are excluded
  from the softmax denominator by an explicit vmask multiply, and masked
  output columns are zero because V is zero there. V_len == 0 cores fall
  back to the reference's uniform-softmax semantics via WK = 0 + vmask = 1.
  Q_len row masking rides the V PSUM->SBUF copy as a per-partition scale.
"""

import numpy as np
import ml_dtypes

B, S, EMB = 8, 4096, 512
H, D = 8, 64
NCORES = 8
KC = EMB // 128          # 4 contraction chunks
NCHUNK = S // 128        # 32 token chunks
SUP = 8                  # token chunks per super-chunk (input DMA granularity)
NSUP = NCHUNK // SUP

_CACHE = {}


def _build(cfg=""):
    import concourse.bacc as bacc
    import concourse.mybir as mybir
    from concourse.tile import TileContext

    f32 = mybir.dt.float32
    f32r = mybir.dt.float32r
    f16 = mybir.dt.float16
    bf16 = mybir.dt.bfloat16
    AX = mybir.AxisListType
    OP = mybir.AluOpType
    ACTF = mybir.ActivationFunctionType

    nc = bacc.Bacc()

    qT = nc.declare_dram_parameter("qT", [EMB, S], f32r, isOutput=False)
    kT = nc.declare_dram_parameter("kT", [EMB, S], f32r, isOutput=False)
    vT = nc.declare_dram_parameter("vT", [EMB, S], f16, isOutput=False)
    wq = nc.declare_dram_parameter("wq", [EMB, EMB], f32r, isOutput=False)
    wk = nc.declare_dram_parameter("wk", [EMB, EMB], f32r, isOutput=False)
    wv = nc.declare_dram_parameter("wv", [EMB, EMB], f16, isOutput=False)
    vmask = nc.declare_dram_parameter("vmask", [128, EMB], bf16, isOutput=False)
    qmask = nc.declare_dram_parameter("qmask", [128, NCHUNK], f32, isOutput=False)
    out = nc.declare_dram_parameter("out", [S, EMB], bf16, isOutput=True)

    def view_hd(ap):
        return ap.rearrange("p (h d) -> p h d", d=D)

    def bcast_hd(ap):
        return ap.rearrange("p (h o) -> p h o", o=1).broadcast_to((128, H, D))

    with TileContext(nc) as tc:
        with (
            tc.tile_pool(name="consts", bufs=1) as cpool,
            tc.tile_pool(name="xin", bufs=2) as xpool,
            tc.tile_pool(name="ps", bufs=2, space="PSUM") as ppool,
            tc.tile_pool(name="work", bufs=4) as wpool,
            tc.tile_pool(name="outw", bufs=2) as opool,
            tc.tile_pool(name="stats", bufs=4) as spool,
        ):
            w_sb = {}
            for name, src, dt_ in (("wq", wq, f32r), ("wk", wk, f32r),
                                   ("wv", wv, f16)):
                tiles = []
                for kc in range(KC):
                    t = cpool.tile([128, EMB], dt_, tag=f"{name}{kc}")
                    nc.sync.dma_start(out=t[:], in_=src[kc * 128:(kc + 1) * 128, :])
                    tiles.append(t)
                w_sb[name] = tiles
            vm_sb = cpool.tile([128, EMB], bf16, tag="vmask")
            nc.sync.dma_start(out=vm_sb[:], in_=vmask[:, :])
            qm_sb = cpool.tile([128, NCHUNK], f32, tag="qm")
            nc.sync.dma_start(out=qm_sb[:], in_=qmask[:, :])

            for s in range(NSUP):
                tok0 = s * SUP * 128
                xs = {}
                for name, src, dt_ in (("q", qT, f32r), ("k", kT, f32r),
                                       ("v", vT, f16)):
                    tiles = []
                    for kc in range(KC):
                        t = xpool.tile([128, SUP * 128], dt_, tag=f"x{name}{kc}")
                        nc.sync.dma_start(
                            out=t[:],
                            in_=src[kc * 128:(kc + 1) * 128, tok0:tok0 + SUP * 128],
                        )
                        tiles.append(t)
                    xs[name] = tiles

                for jp in range(SUP // 4):
                    ow = opool.tile([128, 4 * EMB], bf16, tag="ow")
                    for j2 in range(4):
                        j = jp * 4 + j2
                        chunk = s * SUP + j
                        js = slice(j * 128, (j + 1) * 128)

                        psq = ppool.tile([128, EMB], f32, tag="psq")
                        psk = ppool.tile([128, EMB], f32, tag="psk")
                        psv = ppool.tile([128, EMB], f32, tag="psv")
                        for name, ps, wn in (("k", psk, "wk"), ("v", psv, "wv"),
                                             ("q", psq, "wq")):
                            for kc in range(KC):
                                nc.tensor.matmul(
                                    ps[:],
                                    xs[name][kc][:, js],
                                    w_sb[wn][kc][:],
                                    start=(kc == 0),
                                    stop=(kc == KC - 1),
                                )

                        k_sb = wpool.tile([128, EMB], f32, tag="k_sb")
                        nc.scalar.copy(k_sb[:], psk[:])
                        v_sb = wpool.tile([128, EMB], bf16, tag="v_sb")
                        nc.scalar.activation(
                            v_sb[:], psv[:], ACTF.Copy,
                            scale=qm_sb[:, chunk:chunk + 1],
                        )
                        a = wpool.tile([128, EMB], f32, tag="a")
                        nc.vector.tensor_mul(a[:], psq[:], k_sb[:])
                        mneg = spool.tile([128, H], f32, tag="mneg")
                        nc.vector.tensor_reduce(
                            mneg[:], view_hd(a[:]), axis=AX.X, op=OP.max,
                            negate=True,
                        )
                        t_m = wpool.tile([128, EMB], f32, tag="t_m")
                        nc.gpsimd.tensor_add(
                            view_hd(t_m[:]), view_hd(a[:]), bcast_hd(mneg[:])
                        )
                        e = wpool.tile([128, EMB], bf16, tag="e")
                        nc.scalar.activation(e[:], t_m[:], ACTF.Exp)
                        em = wpool.tile([128, EMB], bf16, tag="em")
                        nc.vector.tensor_mul(em[:], e[:], vm_sb[:])
                        ssum = spool.tile([128, H], f32, tag="ssum")
                        nc.vector.tensor_reduce(
                            ssum[:], view_hd(em[:]), axis=AX.X, op=OP.add
                        )
                        r = spool.tile([128, H], bf16, tag="r")
                        with nc.allow_low_precision(reason="R*E in fp16: 2^-11 rel on softmax weights, well under the 2e-2 gate"):
                            nc.vector.reciprocal(r[:], ssum[:])
                        p = wpool.tile([128, EMB], bf16, tag="p")
                        nc.gpsimd.tensor_mul(
                            view_hd(p[:]), view_hd(em[:]), bcast_hd(r[:])
                        )
                        nc.vector.tensor_mul(
                            ow[:, j2 * EMB:(j2 + 1) * EMB], p[:], v_sb[:]
                        )

                    t0 = (s * SUP + jp * 4) * 128
                    nc.sync.dma_start(
                        out=out[t0:t0 + 512, :].rearrange("(i p) f -> p i f", i=4),
                        in_=ow[:].rearrange("p (i f) -> p i f", i=4),
                    )

    nc.finalize()
    return nc


def _prep_inputs(Q_seq, K_seq, V_seq, Q_len, V_len, WQ, WK, WV):
    in_maps = []
    jpos = np.arange(EMB) % D
    tpos = np.arange(S)
    for b in range(B):
        vl = int(V_len[b, 0])
        ql = int(Q_len[b, 0])
        if vl == 0:
            # Reference semantics collapse to a uniform 1/64 softmax: all
            # logits ride to exactly -1e12 in f32. Reproduce via K = 0
            # (all logits 0 -> uniform) with every position unmasked.
            wk_b = np.zeros_like(WK, dtype=np.float32)
            wv_b = WV.astype(np.float32)
            vmrow = np.ones(EMB, ml_dtypes.bfloat16)
        else:
            keep = (jpos < vl)
            wk_b = np.where(keep[None, :], WK, 0.0).astype(np.float32)
            wv_b = np.where(keep[None, :], WV, 0.0).astype(np.float32)
            vmrow = keep.astype(ml_dtypes.bfloat16)
        vmask = np.broadcast_to(vmrow, (128, EMB)).copy()
        qm = (tpos < ql).astype(np.float32).reshape(NCHUNK, 128).T.copy()
        in_maps.append({
            "qT": np.ascontiguousarray(Q_seq[b].T.astype(np.float32)),
            "kT": np.ascontiguousarray(K_seq[b].T.astype(np.float32)),
            "vT": np.ascontiguousarray(V_seq[b].T.astype(np.float16)),
            "wq": np.ascontiguousarray((WQ * 0.125).astype(np.float32)),
            "wk": np.ascontiguousarray(wk_b),
            "wv": np.ascontiguousarray(wv_b.astype(np.float16)),
            "vmask": vmask,
            "qmask": np.ascontiguousarray(qm),
        })
    return in_maps


def _run(inputs, trace=False, mm_dtype_name="", tmpdir=None):
    from concourse.bass_utils import run_bass_kernel_spmd

    key = "v3"
    if key not in _CACHE:
        _CACHE[key] = _build()
    nc = _CACHE[key]

    in_maps = _prep_inputs(**inputs)
    res = run_bass_kernel_spmd(nc, in_maps, core_ids=list(range(NCORES)),
                               trace=trace, tmpdir=tmpdir)
    out = np.stack([res.results[i]["out"] for i in range(NCORES)], axis=0)
    return out.astype(np.float32), res


def kernel(Q_seq, K_seq, V_seq, Q_len, V_len, WQ, WK, WV):
    out, _ = _run(dict(Q_seq=Q_seq, K_seq=K_seq, V_seq=V_seq,
                       Q_len=Q_len, V_len=V_len, WQ=WQ, WK=WK, WV=WV))
    return out
